# revision 1
# baseline (speedup 1.0000x reference)
"""CeNN layer (nn_CeNNLayer) Trainium2 Bass kernel.

Problem: x [16,64,128,128] f32; per image:
    ic    = conv3x3(x, B_w) + B_b + Z
    s0    = conv3x3(x, rescale_w) + rescale_b
    s_{k+1} = s_k + 0.1*(-s_k + conv3x3(nonlin(s_k), A_w) + A_b + ic),  10 iters
    out   = nonlin(s_10)
with nonlin(v) = max(min(v, 1+a(v-1)), -1+a(min(v,1+a(v-1))+1)), a=0.01.

Sharding: data-parallel over batch, 2 images per NeuronCore on 8 cores.

Per-core layout ("quadrant pixel-split"): every per-pixel tensor is
[128 partitions, ...] where partition p<64 holds channel p of image rows 0-63
(half A) and partition p>=64 holds channel p-64 of rows 64-127 (half B).
The 3x3 conv runs as 9 accumulating K=64 fp16 matmuls per 4-row tile on the
PE-array quadrants (0,0) (half A) and (64,64) (half B) concurrently, into one
[128,512] fp32 psum bank. Cross-half halo rows are exchanged by SBUF-SBUF DMA.

nonlin is computed as  z := nonlin(s)+1 = Lrelu(2 - Lrelu(1 - s)),  slope a,
on the Scalar engine; z is stored padded with pad value 1.0 (z=nl+1 makes the
reference's zero padding a constant), so padding folds into a per-channel bias
vb = 0.1*(B_b+Z+A_b) - 0.1*sum(A_w).  State update s' = 0.9 s + psum + IC is
two fused Vector-engine ops.  z is double-buffered across iterations (stencil
hazard) and the per-iteration tile order rotates by 2 so producers of the next
iteration's first tiles always land early.
"""
import numpy as np

import concourse.bacc as bacc
import concourse.mybir as mybir
import concourse.tile as tile
from concourse.bass_utils import run_bass_kernel_spmd

F32 = mybir.dt.float32
F16 = mybir.dt.float16

ALPHA = 0.01
N_CORES = 8
NIMG = 2            # images per core (batch 16 / 8 cores)
ROWS = 66           # buffer rows: 1 pad/halo + 64 data + 1 pad/halo
PITCH = 130         # 1 pad col + 128 data cols + 1 pad col
NT = 16             # 4-row tiles per half-image
NPIX = 64 * 128     # pixels per half-image
ITERS = 10
TAPS = [(dy, dx) for dy in (-1, 0, 1) for dx in (-1, 0, 1)]

_NC_CACHE = None


def build_nc():
    nc = bacc.Bacc(None, target_bir_lowering=False)

    xp_d = nc.dram_tensor("xp", [128, NIMG, ROWS, PITCH], F16, kind="ExternalInput")
    wt_d = nc.dram_tensor("wt", [128, 27 * 64], F16, kind="ExternalInput")
    bias_d = nc.dram_tensor("bias", [128, 2], F32, kind="ExternalInput")
    yo_d = nc.dram_tensor("yo", [128, NIMG, NPIX], F32, kind="ExternalOutput")

    with tile.TileContext(nc) as tc:
        with (
            tc.tile_pool(name="main", bufs=1) as main,
            tc.tile_pool(name="scr", bufs=6) as scr,
            tc.tile_pool(name="psc", bufs=5, space="PSUM") as psc,
            tc.tile_pool(name="psi", bufs=3, space="PSUM") as psi,
        ):
            xt = main.tile([128, NIMG, ROWS, PITCH], F16)
            zA = main.tile([128, ROWS, PITCH], F16)
            zB = main.tile([128, ROWS, PITCH], F16)
            zbufs = [zA, zB]
            stA = main.tile([128, NPIX], F32)
            stB = main.tile([128, NPIX], F32)
            st_bufs = [stA, stB]
            ict = main.tile([128, NPIX], F32)
            wt = main.tile([128, 27 * 64], F16)
            bt = main.tile([128, 2], F32)
            b1 = main.tile([128, 1], F32)
            b2 = main.tile([128, 1], F32)

            nc.sync.dma_start(wt[:], wt_d[:])
            nc.sync.dma_start(bt[:], bias_d[:])
            nc.sync.dma_start(xt[:, 0:1, 0:21, :], xp_d[:, 0:1, 0:21, :])
            nc.sync.dma_start(xt[:, 0:1, 21:ROWS, :], xp_d[:, 0:1, 21:ROWS, :])
            nc.sync.dma_start(xt[:, 1:2, :, :], xp_d[:, 1:2, :, :])
            nc.gpsimd.memset(b1[:], 1.0)
            nc.gpsimd.memset(b2[:], 2.0)
            nc.gpsimd.memset(zA[:], 1.0)
            nc.gpsimd.memset(zB[:], 1.0)

            LR = mybir.ActivationFunctionType.Lrelu
            ID = mybir.ActivationFunctionType.Identity

            def conv9(psum, wblk, rhs_fn, t):
                # accumulate 9 taps into psum for 4-row tile t, both quadrants
                r0 = 1 + 4 * t
                for j, (dy, dx) in enumerate(TAPS):
                    for pb in (0, 64):
                        nc.tensor.matmul(
                            psum[pb:pb + 64, :],
                            wt[pb:pb + 64, (wblk + j) * 64:(wblk + j + 1) * 64],
                            rhs_fn(pb, r0 + dy, 1 + dx),
                            start=(j == 0),
                            stop=(j == 8),
                            tile_position=(pb, pb),
                            skip_group_check=True,
                        )

            for img in range(NIMG):
                st = st_bufs[img]
                def xrhs(pb, r, c, img=img):
                    return xt[pb:pb + 64, img:img + 1, r:r + 4, c:c + 128]

                def zrhs_for(zt):
                    def zrhs(pb, r, c):
                        return zt[pb:pb + 64, r:r + 4, c:c + 128]
                    return zrhs

                # setup: state0 = conv(x, rescale)+rescale_b; IC = 0.1conv(x,B)+vb
                g0 = img * (ITERS + 1)
                ss = (2 * g0) % NT
                for t in [(ss + i) % NT for i in range(NT)]:
                    off = 512 * t
                    pr = psc.tile([128, 512], F32, tag="conv")
                    pi = psi.tile([128, 512], F32, tag="ic")
                    conv9(pr, 0, xrhs, t)
                    conv9(pi, 9, xrhs, t)
                    nc.scalar.activation(st[:, off:off + 512], pr[:], ID,
                                         bias=bt[:, 0:1], scale=1.0)
                    nc.vector.tensor_scalar(ict[:, off:off + 512], pi[:],
                                            bt[:, 1:2], None,
                                            mybir.AluOpType.add)
                    # z0 = Lrelu(2 - Lrelu(1 - state0)) = nonlin(state0) + 1
                    u = scr.tile([128, 512], F32, tag="u")
                    nc.scalar.activation(u[:], st[:, off:off + 512], LR,
                                         bias=b1[:], scale=-1.0, alpha=ALPHA)
                    r0 = 1 + 4 * t
                    nc.scalar.activation(zbufs[0][:, r0:r0 + 4, 1:129], u[:], LR,
                                         bias=b2[:], scale=-1.0, alpha=ALPHA)
                    if t == NT - 1:
                        # half B top halo <- half A last data row
                        nc.sync.dma_start(zbufs[0][64:128, 0, :], zbufs[0][0:64, 64, :])
                    if t == 0:
                        # half A bottom halo <- half B first data row
                        nc.sync.dma_start(zbufs[0][0:64, 65, :], zbufs[0][64:128, 1, :])

                for it in range(1, ITERS + 1):
                    last = it == ITERS
                    zprev = zbufs[(it + 1) % 2]
                    znext = zbufs[it % 2]
                    s = (2 * (g0 + it)) % NT
                    for t in [(s + i) % NT for i in range(NT)]:
                        off = 512 * t
                        p = psc.tile([128, 512], F32, tag="conv")
                        conv9(p, 18, zrhs_for(zprev), t)
                        tmp = scr.tile([128, 512], F32, tag="tmp")
                        # tmp = 0.9*state + psum ; state = tmp + IC
                        nc.vector.scalar_tensor_tensor(
                            out=tmp[:], in0=st[:, off:off + 512], scalar=0.9,
                            in1=p[:], op0=mybir.AluOpType.mult,
                            op1=mybir.AluOpType.add)
                        nc.vector.tensor_tensor(
                            st[:, off:off + 512], tmp[:], ict[:, off:off + 512],
                            mybir.AluOpType.add)
                        u = scr.tile([128, 512], F32, tag="u")
                        nc.scalar.activation(u[:], st[:, off:off + 512], LR,
                                             bias=b1[:], scale=-1.0, alpha=ALPHA)
                        if not last:
                            r0 = 1 + 4 * t
                            nc.scalar.activation(znext[:, r0:r0 + 4, 1:129], u[:], LR,
                                                 bias=b2[:], scale=-1.0, alpha=ALPHA)
                            if t == NT - 1:
                                nc.sync.dma_start(znext[64:128, 0, :], znext[0:64, 64, :])
                            if t == 0:
                                nc.sync.dma_start(znext[0:64, 65, :], znext[64:128, 1, :])
                        else:
                            zf = scr.tile([128, 512], F32, tag="zf")
                            nc.scalar.activation(zf[:], u[:], LR,
                                                 bias=b2[:], scale=-1.0, alpha=ALPHA)
                            nc.vector.tensor_scalar(
                                st[:, off:off + 512], zf[:], -1.0, None,
                                mybir.AluOpType.add)

                for oc in range(4):
                    nc.sync.dma_start(yo_d[:, img, oc * 2048:(oc + 1) * 2048],
                                      st[:, oc * 2048:(oc + 1) * 2048])

    nc.compile()
    return nc


def pack_inputs(x, rescale_w, rescale_b, A_w, A_b, B_w, B_b, Z, n_cores=N_CORES):
    """Host-side prep: pad/split x per core, build fp16 lhsT tap blocks, biases."""
    x = np.asarray(x, dtype=np.float32)

    def lhsT_blocks(w):  # [co,ci,3,3] -> [64, 9*64] fp16, cols = tap-major, co
        out = np.empty((64, 9 * 64), dtype=np.float16)
        for j, (dy, dx) in enumerate(TAPS):
            out[:, j * 64:(j + 1) * 64] = w[:, :, dy + 1, dx + 1].T.astype(np.float16)
        return out

    wt = np.zeros((128, 27 * 64), dtype=np.float16)
    half = np.concatenate(
        [lhsT_blocks(np.asarray(rescale_w)),
         lhsT_blocks(0.1 * np.asarray(B_w)),
         lhsT_blocks(0.1 * np.asarray(A_w))], axis=1)
    wt[0:64] = half
    wt[64:128] = half

    # vb = 0.1(B_b+Z+A_b) - CA, CA = sum of the fp16 A-taps actually used
    A16 = wt[0:64, 18 * 64:27 * 64].astype(np.float32).reshape(64, 9, 64)
    CA = A16.sum(axis=(0, 1))
    vb = (0.1 * (np.asarray(B_b) + np.asarray(Z) + np.asarray(A_b)) - CA).astype(np.float32)
    bias = np.zeros((128, 2), dtype=np.float32)
    bias[0:64, 0] = rescale_b
    bias[64:128, 0] = rescale_b
    bias[0:64, 1] = vb
    bias[64:128, 1] = vb

    in_maps = []
    for c in range(n_cores):
        xp = np.zeros((128, NIMG, ROWS, PITCH), dtype=np.float16)
        for i in range(NIMG):
            g = x[c * NIMG + i]  # [64, 128, 128]
            xp[0:64, i, 1:65, 1:129] = g[:, 0:64, :]
            xp[0:64, i, 65, 1:129] = g[:, 64, :]
            xp[64:128, i, 1:65, 1:129] = g[:, 64:128, :]
            xp[64:128, i, 0, 1:129] = g[:, 63, :]
        in_maps.append({"xp": xp, "wt": wt, "bias": bias})
    return in_maps


def unpack_outputs(results, n_cores=N_CORES):
    out = np.empty((n_cores * NIMG, 64, 128, 128), dtype=np.float32)
    for c in range(n_cores):
        yo = results[c]["yo"]  # [128, NIMG, NPIX]
        for i in range(NIMG):
            out[c * NIMG + i, :, 0:64, :] = yo[0:64, i].reshape(64, 64, 128)
            out[c * NIMG + i, :, 64:128, :] = yo[64:128, i].reshape(64, 64, 128)
    return out


def kernel(x, rescale_w, rescale_b, A_w, A_b, B_w, B_b, Z, **_):
    global _NC_CACHE
    if _NC_CACHE is None:
        _NC_CACHE = build_nc()
    in_maps = pack_inputs(x, rescale_w, rescale_b, A_w, A_b, B_w, B_b, Z)
    res = run_bass_kernel_spmd(_NC_CACHE, in_maps, list(range(N_CORES)))
    return unpack_outputs(res.results)



# revision 4
# speedup vs baseline: 1.4107x; 1.4107x over previous
"""CeNN layer (nn_CeNNLayer) Trainium2 Bass kernel — column-parity packed conv.

Problem: x [16,64,128,128] f32; per image:
    ic    = 0.1*(conv3x3(x, B_w) + B_b + Z)
    s0    = conv3x3(x, rescale_w) + rescale_b
    s_{k+1} = 0.9 s_k + 0.1*(conv3x3(nonlin(s_k), A_w) + A_b) + ic,  10 iters
    out   = nonlin(s_10)

Sharding: data-parallel over batch, 2 images per NeuronCore on 8 cores.

Per-core layout ("column-parity split"): partition p<64 holds channel p of the
EVEN pixel columns, partition p>=64 holds channel p-64 of the ODD columns.
Image rows live in the free dimension, so no cross-partition halo exchange is
needed; row/col pads are part of each buffer (pad value 1.0 in z-space).

The 3x3 conv needs only 6 matmul slots per psum bank (vs 9 for the naive
per-tap schedule): for each dy, one "dense" slot packs taps (dx=0 via even
data, dx=+1 via odd) into a K=128 matmul, and one "edge" slot covers the
remaining taps at shifted offsets with the unused K-half zero-weighted.
Every matmul runs in 128x64 column-tiling mode: tile (0,0) produces even
outputs (psum partitions 0-63), tile (0,64) odd outputs, concurrently, so the
PE array is 100% utilized during dense slots and 50% during edge slots
(structural 75% utilization vs the 50% of a 2-quadrant per-tap kernel).

State updates run in-place on the Vector engine in 2048-wide chunks (4 psum
banks) to amortize per-op overheads; nonlin z = Lrelu(2 - Lrelu(1 - s)) on the
Scalar engine likewise.  The two images interleave at the group level so one
image's evac/nonlin tail hides under the other image's convs.
"""
import numpy as np

import concourse.bacc as bacc
import concourse.mybir as mybir
import concourse.tile as tile
from concourse.bass_utils import run_bass_kernel_spmd

F32 = mybir.dt.float32
F16 = mybir.dt.float16

ALPHA = 0.01
N_CORES = 8
NIMG = 2            # images per core (batch 16 / 8 cores)
ROWS = 130          # 1 pad row + 128 data rows + 1 pad row
PITCH = 66          # 1 pad pair + 64 data pairs + 1 pad pair
NPIX = 128 * 64     # free-dim pixels per partition per image (rows x pairs)
ITERS = 10
NG = 4              # row-groups per image (32 rows / 2048 px each)
CHUNK = 2048        # psum tile free size (4 banks)
DYS = (-1, 0, 1)

_NC_CACHE = None


def build_nc():
    nc = bacc.Bacc(None, target_bir_lowering=False)

    xp_d = nc.dram_tensor("xp", [128, NIMG, ROWS, PITCH], F16, kind="ExternalInput")
    wt_d = nc.dram_tensor("wt", [128, 36 * 64], F16, kind="ExternalInput")
    bias_d = nc.dram_tensor("bias", [128, 2], F32, kind="ExternalInput")
    yo_d = nc.dram_tensor("yo", [128, NIMG, NPIX], F32, kind="ExternalOutput")

    LR = mybir.ActivationFunctionType.Lrelu
    ID = mybir.ActivationFunctionType.Identity

    with tile.TileContext(nc) as tc:
        with (
            tc.tile_pool(name="main", bufs=1) as main,
            tc.tile_pool(name="xg", bufs=2) as xpool,
            tc.tile_pool(name="scr", bufs=2) as scr,
            tc.tile_pool(name="ps", bufs=2, space="PSUM") as psp,
        ):
            zt = [[main.tile([128, ROWS, PITCH], F16, name=f"z{i}{k}", tag=f"z{i}{k}")
                   for k in range(2)] for i in range(NIMG)]
            st = [main.tile([128, NPIX], F32, name=f"st{i}", tag=f"st{i}") for i in range(NIMG)]
            ict = [main.tile([128, NPIX], F16, name=f"ic{i}", tag=f"ic{i}") for i in range(NIMG)]
            wt = main.tile([128, 36 * 64], F16)
            bt = main.tile([128, 2], F32)
            b1 = main.tile([128, 1], F32)
            b2 = main.tile([128, 1], F32)

            nc.sync.dma_start(wt[:], wt_d[:])
            nc.sync.dma_start(bt[:], bias_d[:])
            nc.gpsimd.memset(b1[:], 1.0)
            nc.gpsimd.memset(b2[:], 2.0)
            for img in range(NIMG):
                for k in range(2):
                    nc.gpsimd.memset(zt[img][k][:], 1.0)

            def wb(ci, di, which):
                c0 = ((ci * 3 + di) * 4 + which) * 64
                return wt[:, c0:c0 + 64]

            def conv_group(ps, ci, rhs, g):
                # rhs(dy, b, shift) -> AP; shift in {0: j-1, 1: j, 2: j+1}
                for di, dy in enumerate(DYS):
                    for b in range(4):
                        off = 512 * b
                        nc.tensor.matmul(
                            ps[0:64, off:off + 512], wb(ci, di, 0), rhs(dy, b, 1),
                            start=(di == 0), stop=False,
                            tile_position=(0, 0), skip_group_check=True)
                        nc.tensor.matmul(
                            ps[64:128, off:off + 512], wb(ci, di, 1), rhs(dy, b, 1),
                            start=(di == 0), stop=False,
                            tile_position=(0, 64), skip_group_check=True)
                for di, dy in enumerate(DYS):
                    for b in range(4):
                        off = 512 * b
                        nc.tensor.matmul(
                            ps[0:64, off:off + 512], wb(ci, di, 2), rhs(dy, b, 0),
                            start=False, stop=(di == 2),
                            tile_position=(0, 0), skip_group_check=True)
                        nc.tensor.matmul(
                            ps[64:128, off:off + 512], wb(ci, di, 3), rhs(dy, b, 2),
                            start=False, stop=(di == 2),
                            tile_position=(0, 64), skip_group_check=True)

            def zrhs(zsrc, g):
                def rhs(dy, b, shift):
                    r0 = 32 * g + 8 * b + dy + 1
                    return zsrc[:, r0:r0 + 8, shift:shift + 64]
                return rhs

            def xrhs(xg):
                def rhs(dy, b, shift):
                    r0 = 8 * b + dy + 1
                    return xg[:, r0:r0 + 8, shift:shift + 64]
                return rhs

            # ---- setup: s0 = conv(x, rescale)+b; ic = 0.1 conv(x, B)+vb; z0 ----
            for img in range(NIMG):
                for g in range(NG):
                    off = CHUNK * g
                    xg = xpool.tile([128, 34, PITCH], F16, tag="xg")
                    nc.sync.dma_start(xg[:], xp_d[:, img, 32 * g:32 * g + 34, :])
                    psA = psp.tile([128, CHUNK], F32, tag="ps")
                    conv_group(psA, 0, xrhs(xg), g)
                    psB = psp.tile([128, CHUNK], F32, tag="ps")
                    conv_group(psB, 1, xrhs(xg), g)
                    nc.scalar.activation(st[img][:, off:off + CHUNK], psA[:], ID,
                                         bias=bt[:, 0:1], scale=1.0)
                    nc.vector.tensor_scalar(ict[img][:, off:off + CHUNK], psB[:],
                                            bt[:, 1:2], None, mybir.AluOpType.add)
                    u = scr.tile([128, CHUNK], F32, tag="u")
                    nc.scalar.activation(u[:], st[img][:, off:off + CHUNK], LR,
                                         bias=b1[:], scale=-1.0, alpha=ALPHA)
                    nc.scalar.activation(zt[img][0][:, 1 + 32 * g:33 + 32 * g, 1:65],
                                         u[:], LR, bias=b2[:], scale=-1.0, alpha=ALPHA)

            # ---- iterations ----
            for it in range(1, ITERS + 1):
                last = it == ITERS
                for img in range(NIMG):
                    zprev = zt[img][(it - 1) % 2]
                    znext = zt[img][it % 2]
                    for g in range(NG):
                        off = CHUNK * g
                        ssl = st[img][:, off:off + CHUNK]
                        ps = psp.tile([128, CHUNK], F32, tag="ps")
                        conv_group(ps, 2, zrhs(zprev, g), g)
                        nc.vector.scalar_tensor_tensor(
                            out=ssl, in0=ssl, scalar=0.9, in1=ps[:],
                            op0=mybir.AluOpType.mult, op1=mybir.AluOpType.add)
                        nc.vector.tensor_tensor(
                            ssl, ssl, ict[img][:, off:off + CHUNK],
                            mybir.AluOpType.add)
                        u = scr.tile([128, CHUNK], F32, tag="u")
                        nc.scalar.activation(u[:], ssl, LR,
                                             bias=b1[:], scale=-1.0, alpha=ALPHA)
                        if not last:
                            nc.scalar.activation(
                                znext[:, 1 + 32 * g:33 + 32 * g, 1:65], u[:], LR,
                                bias=b2[:], scale=-1.0, alpha=ALPHA)
                        else:
                            nc.scalar.activation(u[:], u[:], LR,
                                                 bias=b2[:], scale=-1.0, alpha=ALPHA)
                            nc.vector.tensor_scalar(ssl, u[:], -1.0, None,
                                                    mybir.AluOpType.add)
                            nc.sync.dma_start(yo_d[:, img, off:off + CHUNK], ssl)

    nc.compile()
    return nc


def pack_inputs(x, rescale_w, rescale_b, A_w, A_b, B_w, B_b, Z, n_cores=N_CORES):
    """Host-side prep: parity-pack x per core, build fp16 lhsT blocks, biases."""
    x = np.asarray(x, dtype=np.float32)

    def blocks(w):  # w [o,c,3,3] f32 -> [128, 12*64] f16 lhsT blocks
        out = np.zeros((128, 12 * 64), dtype=np.float16)
        for di, dy in enumerate((-1, 0, 1)):
            b0 = di * 4 * 64
            c = w[:, :, dy + 1, 1].T.astype(np.float16)  # dx=0
            l = w[:, :, dy + 1, 0].T.astype(np.float16)  # dx=-1
            r = w[:, :, dy + 1, 2].T.astype(np.float16)  # dx=+1
            out[0:64, b0:b0 + 64] = c          # T0 dense: even data, dx=0
            out[64:128, b0:b0 + 64] = r        # T0 dense: odd data, dx=+1
            out[0:64, b0 + 64:b0 + 128] = l    # T1 dense: even data, dx=-1
            out[64:128, b0 + 64:b0 + 128] = c  # T1 dense: odd data, dx=0
            out[64:128, b0 + 128:b0 + 192] = l  # T0 edge: odd data @ j-1, dx=-1
            out[0:64, b0 + 192:b0 + 256] = r    # T1 edge: even data @ j+1, dx=+1
        return out

    wt = np.concatenate([
        blocks(np.asarray(rescale_w, np.float32)),
        blocks(0.1 * np.asarray(B_w, np.float32)),
        blocks(0.1 * np.asarray(A_w, np.float32)),
    ], axis=1)

    # vb = 0.1(B_b+Z+A_b) - CA;  CA = per-out-channel sum of fp16 A taps used
    A16 = (0.1 * np.asarray(A_w, np.float32)).astype(np.float16).astype(np.float32)
    CA = A16.sum(axis=(1, 2, 3))
    vb = (0.1 * (np.asarray(B_b) + np.asarray(Z) + np.asarray(A_b)) - CA).astype(np.float32)
    bias = np.zeros((128, 2), dtype=np.float32)
    bias[0:64, 0] = rescale_b
    bias[64:128, 0] = rescale_b
    bias[0:64, 1] = vb
    bias[64:128, 1] = vb

    in_maps = []
    for c in range(n_cores):
        xp = np.zeros((128, NIMG, ROWS, PITCH), dtype=np.float16)
        for i in range(NIMG):
            g = x[c * NIMG + i]  # [64, 128, 128]
            xp[0:64, i, 1:129, 1:65] = g[:, :, 0::2]
            xp[64:128, i, 1:129, 1:65] = g[:, :, 1::2]
        in_maps.append({"xp": xp, "wt": wt, "bias": bias})
    return in_maps


def unpack_outputs(results, n_cores=N_CORES):
    out = np.empty((n_cores * NIMG, 64, 128, 128), dtype=np.float32)
    for c in range(n_cores):
        yo = results[c]["yo"].reshape(128, NIMG, 128, 64)
        for i in range(NIMG):
            out[c * NIMG + i, :, :, 0::2] = yo[0:64, i]
            out[c * NIMG + i, :, :, 1::2] = yo[64:128, i]
    return out


def kernel(x, rescale_w, rescale_b, A_w, A_b, B_w, B_b, Z, **_):
    global _NC_CACHE
    if _NC_CACHE is None:
        _NC_CACHE = build_nc()
    in_maps = pack_inputs(x, rescale_w, rescale_b, A_w, A_b, B_w, B_b, Z)
    res = run_bass_kernel_spmd(_NC_CACHE, in_maps, list(range(N_CORES)))
    return unpack_outputs(res.results)


# revision 10
# speedup vs baseline: 1.4804x; 1.0494x over previous
"""CeNN layer (nn_CeNNLayer) Trainium2 Bass kernel — column-parity packed conv.

Problem: x [16,64,128,128] f32; per image:
    ic    = 0.1*(conv3x3(x, B_w) + B_b + Z)
    s0    = conv3x3(x, rescale_w) + rescale_b
    s_{k+1} = 0.9 s_k + 0.1*(conv3x3(nonlin(s_k), A_w) + A_b) + ic,  10 iters
    out   = nonlin(s_10)

Sharding: data-parallel over batch, 2 images per NeuronCore on 8 cores.

Per-core layout ("column-parity split"): partition p<64 holds channel p of the
EVEN pixel columns, partition p>=64 holds channel p-64 of the ODD columns.
Image rows live in the free dimension, so no cross-partition halo exchange is
needed; row/col pads are part of each buffer (pad value 1.0 in z-space).

The 3x3 conv needs only 6 matmul slots per psum bank (vs 9 for the naive
per-tap schedule): for each dy, one "dense" slot packs taps (dx=0 via even
data, dx=+1 via odd) into a K=128 matmul, and one "edge" slot covers the
remaining taps at shifted offsets with the unused K-half zero-weighted.
Every matmul runs in 128x64 column-tiling mode: tile (0,0) produces even
outputs (psum partitions 0-63), tile (0,64) odd outputs, concurrently, so the
PE array is 100% utilized during dense slots and 50% during edge slots
(structural 75% utilization vs the 50% of a 2-quadrant per-tap kernel).

State updates run in-place on the Vector engine in 2048-wide chunks (4 psum
banks) to amortize per-op overheads; nonlin z = Lrelu(2 - Lrelu(1 - s)) on the
Scalar engine likewise.  The two images interleave at the group level so one
image's evac/nonlin tail hides under the other image's convs.
"""
import numpy as np

import concourse.bacc as bacc
import concourse.mybir as mybir
import concourse.tile as tile
from concourse.bass_utils import run_bass_kernel_spmd

F32 = mybir.dt.float32
F16 = mybir.dt.float16

ALPHA = 0.01
N_CORES = 8
NIMG = 2            # images per core (batch 16 / 8 cores)
ROWS = 130          # 1 pad row + 128 data rows + 1 pad row
PITCH = 66          # 1 pad pair + 64 data pairs + 1 pad pair
NPIX = 128 * 64     # free-dim pixels per partition per image (rows x pairs)
ITERS = 10
NG = 4              # row-groups per image (32 rows / 2048 px each)
CHUNK = 2048        # psum tile free size (4 banks)
DYS = (-1, 0, 1)

_NC_CACHE = None


def build_nc():
    nc = bacc.Bacc(None, target_bir_lowering=False)

    xp_d = nc.dram_tensor("xp", [128, NIMG, ROWS, PITCH], F16, kind="ExternalInput")
    wt_d = nc.dram_tensor("wt", [128, 38 * 64], F16, kind="ExternalInput")
    bias_d = nc.dram_tensor("bias", [128, 2], F32, kind="ExternalInput")
    yo_d = nc.dram_tensor("yo", [128, NIMG, NPIX], F32, kind="ExternalOutput")

    LR = mybir.ActivationFunctionType.Lrelu
    ID = mybir.ActivationFunctionType.Identity

    with tile.TileContext(nc) as tc:
        with (
            tc.tile_pool(name="main", bufs=1) as main,
            tc.tile_pool(name="xg", bufs=2) as xpool,
            tc.tile_pool(name="scr", bufs=2) as scr,
            tc.tile_pool(name="ps", bufs=2, space="PSUM") as psp,
        ):
            zt = [[main.tile([128, ROWS, PITCH], F16, name=f"z{i}{k}", tag=f"z{i}{k}")
                   for k in range(2)] for i in range(NIMG)]
            st = [main.tile([128, NPIX], F32, name=f"st{i}", tag=f"st{i}") for i in range(NIMG)]
            ict = [main.tile([128, NPIX], F16, name=f"ic{i}", tag=f"ic{i}") for i in range(NIMG)]
            wt = main.tile([128, 38 * 64], F16)
            bt = main.tile([128, 2], F32)
            b1 = main.tile([128, 1], F32)
            b2 = main.tile([128, 1], F32)

            nc.sync.dma_start(wt[:], wt_d[:])
            nc.sync.dma_start(bt[:], bias_d[:])
            nc.gpsimd.memset(b1[:], 1.0)
            nc.gpsimd.memset(b2[:], 2.0)
            for img in range(NIMG):
                for k in range(2):
                    nc.gpsimd.memset(zt[img][k][:], 1.0)

            def wb(ci, di, which):
                c0 = ((ci * 3 + di) * 4 + which) * 64
                return wt[:, c0:c0 + 64]

            def conv_group(ps, ci, rhs, g, ic_rhs=None):
                # rhs(dy, b, shift) -> AP; shift in {0: j-1, 1: j, 2: j+1}
                # ic_rhs(b) -> AP: optional fp16 tensor added via identity matmul
                fin = ic_rhs is None
                for di, dy in enumerate(DYS):
                    for b in range(4):
                        off = 512 * b
                        nc.tensor.matmul(
                            ps[0:64, off:off + 512], wb(ci, di, 0), rhs(dy, b, 1),
                            start=(di == 0), stop=False,
                            tile_position=(0, 0), skip_group_check=True)
                        nc.tensor.matmul(
                            ps[64:128, off:off + 512], wb(ci, di, 1), rhs(dy, b, 1),
                            start=(di == 0), stop=False,
                            tile_position=(0, 64), skip_group_check=True)
                for di, dy in enumerate(DYS):
                    for b in range(4):
                        off = 512 * b
                        nc.tensor.matmul(
                            ps[0:64, off:off + 512], wb(ci, di, 2), rhs(dy, b, 0),
                            start=False, stop=(fin and di == 2),
                            tile_position=(0, 0), skip_group_check=True)
                        nc.tensor.matmul(
                            ps[64:128, off:off + 512], wb(ci, di, 3), rhs(dy, b, 2),
                            start=False, stop=(fin and di == 2),
                            tile_position=(0, 64), skip_group_check=True)
                if ic_rhs is not None:
                    for b in range(4):
                        off = 512 * b
                        nc.tensor.matmul(
                            ps[0:64, off:off + 512], wt[:, 36 * 64:37 * 64],
                            ic_rhs(b), start=False, stop=True,
                            tile_position=(0, 0), skip_group_check=True)
                        nc.tensor.matmul(
                            ps[64:128, off:off + 512], wt[:, 37 * 64:38 * 64],
                            ic_rhs(b), start=False, stop=True,
                            tile_position=(0, 64), skip_group_check=True)

            def zrhs(zsrc, g):
                def rhs(dy, b, shift):
                    r0 = 32 * g + 8 * b + dy + 1
                    return zsrc[:, r0:r0 + 8, shift:shift + 64]
                return rhs

            def xrhs(xg):
                def rhs(dy, b, shift):
                    r0 = 8 * b + dy + 1
                    return xg[:, r0:r0 + 8, shift:shift + 64]
                return rhs

            # ---- setup: s0 = conv(x, rescale)+b; ic = 0.1 conv(x, B)+vb; z0 ----
            for img in range(NIMG):
                for g in range(NG):
                    off = CHUNK * g
                    xg = xpool.tile([128, 34, PITCH], F16, tag="xg")
                    nc.sync.dma_start(xg[:], xp_d[:, img, 32 * g:32 * g + 34, :])
                    psA = psp.tile([128, CHUNK], F32, tag="ps")
                    conv_group(psA, 0, xrhs(xg), g)
                    psB = psp.tile([128, CHUNK], F32, tag="ps")
                    conv_group(psB, 1, xrhs(xg), g)
                    nc.scalar.activation(st[img][:, off:off + CHUNK], psA[:], ID,
                                         bias=bt[:, 0:1], scale=1.0)
                    nc.vector.tensor_scalar(ict[img][:, off:off + CHUNK], psB[:],
                                            bt[:, 1:2], None, mybir.AluOpType.add)
                    u = scr.tile([128, CHUNK], F32, tag="u")
                    nc.scalar.activation(u[:], st[img][:, off:off + CHUNK], LR,
                                         bias=b1[:], scale=-1.0, alpha=ALPHA)
                    nc.scalar.activation(zt[img][0][:, 1 + 32 * g:33 + 32 * g, 1:65],
                                         u[:], LR, bias=b2[:], scale=-1.0, alpha=ALPHA)

            # ---- iterations ----
            for it in range(1, ITERS + 1):
                last = it == ITERS
                for img in range(NIMG):
                    zprev = zt[img][(it - 1) % 2]
                    znext = zt[img][it % 2]
                    for g in range(NG):
                        off = CHUNK * g
                        ssl = st[img][:, off:off + CHUNK]
                        ps = psp.tile([128, CHUNK], F32, tag="ps")
                        if not last:
                            conv_group(ps, 2, zrhs(zprev, g), g)
                            nc.vector.scalar_tensor_tensor(
                                out=ssl, in0=ssl, scalar=0.9, in1=ps[:],
                                op0=mybir.AluOpType.mult, op1=mybir.AluOpType.add)
                            nc.vector.tensor_tensor(
                                ssl, ssl, ict[img][:, off:off + CHUNK],
                                mybir.AluOpType.add)
                        else:
                            # fold "+ict" into psum via identity matmuls so the
                            # tail only needs one DVE op per chunk
                            def ic_rhs(b, img=img, off=off):
                                return ict[img][:, off + 512 * b:off + 512 * b + 512]
                            conv_group(ps, 2, zrhs(zprev, g), g, ic_rhs=ic_rhs)
                            nc.vector.scalar_tensor_tensor(
                                out=ssl, in0=ssl, scalar=0.9, in1=ps[:],
                                op0=mybir.AluOpType.mult, op1=mybir.AluOpType.add)
                        u = scr.tile([128, CHUNK], F32, tag="u")
                        nc.scalar.activation(u[:], ssl, LR,
                                             bias=b1[:], scale=-1.0, alpha=ALPHA)
                        if not last:
                            nc.scalar.activation(
                                znext[:, 1 + 32 * g:33 + 32 * g, 1:65], u[:], LR,
                                bias=b2[:], scale=-1.0, alpha=ALPHA)
                        else:
                            # write z-space result; host subtracts the 1
                            nc.scalar.activation(u[:], u[:], LR,
                                                 bias=b2[:], scale=-1.0, alpha=ALPHA)
                            nc.sync.dma_start(yo_d[:, img, off:off + CHUNK], u[:])

    nc.compile()
    return nc


def pack_inputs(x, rescale_w, rescale_b, A_w, A_b, B_w, B_b, Z, n_cores=N_CORES):
    """Host-side prep: parity-pack x per core, build fp16 lhsT blocks, biases."""
    x = np.asarray(x, dtype=np.float32)

    def blocks(w):  # w [o,c,3,3] f32 -> [128, 12*64] f16 lhsT blocks
        out = np.zeros((128, 12 * 64), dtype=np.float16)
        for di, dy in enumerate((-1, 0, 1)):
            b0 = di * 4 * 64
            c = w[:, :, dy + 1, 1].T.astype(np.float16)  # dx=0
            l = w[:, :, dy + 1, 0].T.astype(np.float16)  # dx=-1
            r = w[:, :, dy + 1, 2].T.astype(np.float16)  # dx=+1
            out[0:64, b0:b0 + 64] = c          # T0 dense: even data, dx=0
            out[64:128, b0:b0 + 64] = r        # T0 dense: odd data, dx=+1
            out[0:64, b0 + 64:b0 + 128] = l    # T1 dense: even data, dx=-1
            out[64:128, b0 + 64:b0 + 128] = c  # T1 dense: odd data, dx=0
            out[64:128, b0 + 128:b0 + 192] = l  # T0 edge: odd data @ j-1, dx=-1
            out[0:64, b0 + 192:b0 + 256] = r    # T1 edge: even data @ j+1, dx=+1
        return out

    ident = np.zeros((128, 2 * 64), dtype=np.float16)
    ident[0:64, 0:64] = np.eye(64, dtype=np.float16)      # T0: psum[m] += rhs[m]
    ident[64:128, 64:128] = np.eye(64, dtype=np.float16)  # T1: psum[64+m] += rhs[64+m]
    wt = np.concatenate([
        blocks(np.asarray(rescale_w, np.float32)),
        blocks(0.1 * np.asarray(B_w, np.float32)),
        blocks(0.1 * np.asarray(A_w, np.float32)),
        ident,
    ], axis=1)

    # vb = 0.1(B_b+Z+A_b) - CA;  CA = per-out-channel sum of fp16 A taps used
    A16 = (0.1 * np.asarray(A_w, np.float32)).astype(np.float16).astype(np.float32)
    CA = A16.sum(axis=(1, 2, 3))
    vb = (0.1 * (np.asarray(B_b) + np.asarray(Z) + np.asarray(A_b)) - CA).astype(np.float32)
    bias = np.zeros((128, 2), dtype=np.float32)
    bias[0:64, 0] = rescale_b
    bias[64:128, 0] = rescale_b
    bias[0:64, 1] = vb
    bias[64:128, 1] = vb

    in_maps = []
    for c in range(n_cores):
        xp = np.zeros((128, NIMG, ROWS, PITCH), dtype=np.float16)
        for i in range(NIMG):
            g = x[c * NIMG + i]  # [64, 128, 128]
            xp[0:64, i, 1:129, 1:65] = g[:, :, 0::2]
            xp[64:128, i, 1:129, 1:65] = g[:, :, 1::2]
        in_maps.append({"xp": xp, "wt": wt, "bias": bias})
    return in_maps


def unpack_outputs(results, n_cores=N_CORES):
    out = np.empty((n_cores * NIMG, 64, 128, 128), dtype=np.float32)
    for c in range(n_cores):
        yo = results[c]["yo"].reshape(128, NIMG, 128, 64)
        for i in range(NIMG):
            # device stores z-space (nonlin + 1); undo the shift here
            out[c * NIMG + i, :, :, 0::2] = yo[0:64, i] - 1.0
            out[c * NIMG + i, :, :, 1::2] = yo[64:128, i] - 1.0
    return out


def kernel(x, rescale_w, rescale_b, A_w, A_b, B_w, B_b, Z, **_):
    global _NC_CACHE
    if _NC_CACHE is None:
        _NC_CACHE = build_nc()
    in_maps = pack_inputs(x, rescale_w, rescale_b, A_w, A_b, B_w, B_b, Z)
    res = run_bass_kernel_spmd(_NC_CACHE, in_maps, list(range(N_CORES)))
    return unpack_outputs(res.results)


# revision 11
# speedup vs baseline: 1.4836x; 1.0022x over previous
"""CeNN layer (nn_CeNNLayer) Trainium2 Bass kernel — column-parity packed conv.

Problem: x [16,64,128,128] f32; per image:
    ic    = 0.1*(conv3x3(x, B_w) + B_b + Z)
    s0    = conv3x3(x, rescale_w) + rescale_b
    s_{k+1} = 0.9 s_k + 0.1*(conv3x3(nonlin(s_k), A_w) + A_b) + ic,  10 iters
    out   = nonlin(s_10)

Sharding: data-parallel over batch, 2 images per NeuronCore on 8 cores.

Per-core layout ("column-parity split"): partition p<64 holds channel p of the
EVEN pixel columns, partition p>=64 holds channel p-64 of the ODD columns.
Image rows live in the free dimension, so no cross-partition halo exchange is
needed; row/col pads are part of each buffer (pad value 1.0 in z-space).

The 3x3 conv needs only 6 matmul slots per psum bank (vs 9 for the naive
per-tap schedule): for each dy, one "dense" slot packs taps (dx=0 via even
data, dx=+1 via odd) into a K=128 matmul, and one "edge" slot covers the
remaining taps at shifted offsets with the unused K-half zero-weighted.
Every matmul runs in 128x64 column-tiling mode: tile (0,0) produces even
outputs (psum partitions 0-63), tile (0,64) odd outputs, concurrently, so the
PE array is 100% utilized during dense slots and 50% during edge slots
(structural 75% utilization vs the 50% of a 2-quadrant per-tap kernel).

State updates run in-place on the Vector engine in 2048-wide chunks (4 psum
banks) to amortize per-op overheads; nonlin z = Lrelu(2 - Lrelu(1 - s)) on the
Scalar engine likewise.  The two images interleave at the group level so one
image's evac/nonlin tail hides under the other image's convs.
"""
import numpy as np

import concourse.bacc as bacc
import concourse.mybir as mybir
import concourse.tile as tile
from concourse.bass_utils import run_bass_kernel_spmd

F32 = mybir.dt.float32
F16 = mybir.dt.float16

ALPHA = 0.01
N_CORES = 8
NIMG = 2            # images per core (batch 16 / 8 cores)
ROWS = 130          # 1 pad row + 128 data rows + 1 pad row
PITCH = 66          # 1 pad pair + 64 data pairs + 1 pad pair
NPIX = 128 * 64     # free-dim pixels per partition per image (rows x pairs)
ITERS = 10
NG = 4              # row-groups per image (32 rows / 2048 px each)
CHUNK = 2048        # psum tile free size (4 banks)
DYS = (-1, 0, 1)

_NC_CACHE = None


def build_nc():
    nc = bacc.Bacc(None, target_bir_lowering=False)

    xp_d = nc.dram_tensor("xp", [128, NIMG, ROWS, PITCH], F16, kind="ExternalInput")
    wt_d = nc.dram_tensor("wt", [128, 38 * 64], F16, kind="ExternalInput")
    bias_d = nc.dram_tensor("bias", [128, 2], F32, kind="ExternalInput")
    yo_d = nc.dram_tensor("yo", [128, NIMG, NPIX], F32, kind="ExternalOutput")

    LR = mybir.ActivationFunctionType.Lrelu
    ID = mybir.ActivationFunctionType.Identity

    with tile.TileContext(nc) as tc:
        with (
            tc.tile_pool(name="main", bufs=1) as main,
            tc.tile_pool(name="xg", bufs=2) as xpool,
            tc.tile_pool(name="scr", bufs=2) as scr,
            tc.tile_pool(name="ps", bufs=2, space="PSUM") as psp,
        ):
            zt = [[main.tile([128, ROWS, PITCH], F16, name=f"z{i}{k}", tag=f"z{i}{k}")
                   for k in range(2)] for i in range(NIMG)]
            st = [main.tile([128, NPIX], F32, name=f"st{i}", tag=f"st{i}") for i in range(NIMG)]
            ict = [main.tile([128, NPIX], F16, name=f"ic{i}", tag=f"ic{i}") for i in range(NIMG)]
            wt = main.tile([128, 38 * 64], F16)
            bt = main.tile([128, 2], F32)
            b1 = main.tile([128, 1], F32)
            b2 = main.tile([128, 1], F32)

            nc.sync.dma_start(wt[:], wt_d[:])
            nc.sync.dma_start(bt[:], bias_d[:])
            nc.gpsimd.memset(b1[:], 1.0)
            nc.gpsimd.memset(b2[:], 2.0)
            for img in range(NIMG):
                for k in range(2):
                    nc.gpsimd.memset(zt[img][k][:], 1.0)

            def wb(ci, di, which):
                c0 = ((ci * 3 + di) * 4 + which) * 64
                return wt[:, c0:c0 + 64]

            def conv_group(ps, ci, rhs, g, ic_rhs=None):
                # rhs(dy, b, shift) -> AP; shift in {0: j-1, 1: j, 2: j+1}
                # ic_rhs(b) -> AP: optional fp16 tensor added via identity matmul
                fin = ic_rhs is None
                for di, dy in enumerate(DYS):
                    for b in range(4):
                        off = 512 * b
                        nc.tensor.matmul(
                            ps[0:64, off:off + 512], wb(ci, di, 0), rhs(dy, b, 1),
                            start=(di == 0), stop=False,
                            tile_position=(0, 0), skip_group_check=True)
                        nc.tensor.matmul(
                            ps[64:128, off:off + 512], wb(ci, di, 1), rhs(dy, b, 1),
                            start=(di == 0), stop=False,
                            tile_position=(0, 64), skip_group_check=True)
                for di, dy in enumerate(DYS):
                    for b in range(4):
                        off = 512 * b
                        nc.tensor.matmul(
                            ps[0:64, off:off + 512], wb(ci, di, 2), rhs(dy, b, 0),
                            start=False, stop=(fin and di == 2),
                            tile_position=(0, 0), skip_group_check=True)
                        nc.tensor.matmul(
                            ps[64:128, off:off + 512], wb(ci, di, 3), rhs(dy, b, 2),
                            start=False, stop=(fin and di == 2),
                            tile_position=(0, 64), skip_group_check=True)
                if ic_rhs is not None:
                    for b in range(4):
                        off = 512 * b
                        nc.tensor.matmul(
                            ps[0:64, off:off + 512], wt[:, 36 * 64:37 * 64],
                            ic_rhs(b), start=False, stop=True,
                            tile_position=(0, 0), skip_group_check=True)
                        nc.tensor.matmul(
                            ps[64:128, off:off + 512], wt[:, 37 * 64:38 * 64],
                            ic_rhs(b), start=False, stop=True,
                            tile_position=(0, 64), skip_group_check=True)

            def zrhs(zsrc, g):
                def rhs(dy, b, shift):
                    r0 = 32 * g + 8 * b + dy + 1
                    return zsrc[:, r0:r0 + 8, shift:shift + 64]
                return rhs

            def xrhs(xg):
                def rhs(dy, b, shift):
                    r0 = 8 * b + dy + 1
                    return xg[:, r0:r0 + 8, shift:shift + 64]
                return rhs

            # ---- setup: s0 = conv(x, rescale)+b; ic = 0.1 conv(x, B)+vb; z0 ----
            for img in range(NIMG):
                for g in range(NG):
                    off = CHUNK * g
                    xg = xpool.tile([128, 34, PITCH], F16, tag="xg")
                    nc.sync.dma_start(xg[:], xp_d[:, img, 32 * g:32 * g + 34, :])
                    psA = psp.tile([128, CHUNK], F32, tag="ps")
                    conv_group(psA, 0, xrhs(xg), g)
                    psB = psp.tile([128, CHUNK], F32, tag="ps")
                    conv_group(psB, 1, xrhs(xg), g)
                    nc.scalar.activation(st[img][:, off:off + CHUNK], psA[:], ID,
                                         bias=bt[:, 0:1], scale=1.0)
                    nc.vector.tensor_scalar(ict[img][:, off:off + CHUNK], psB[:],
                                            bt[:, 1:2], None, mybir.AluOpType.add)
                    u = scr.tile([128, CHUNK], F32, tag="u")
                    nc.scalar.activation(u[:], st[img][:, off:off + CHUNK], LR,
                                         bias=b1[:], scale=-1.0, alpha=ALPHA)
                    nc.scalar.activation(zt[img][0][:, 1 + 32 * g:33 + 32 * g, 1:65],
                                         u[:], LR, bias=b2[:], scale=-1.0, alpha=ALPHA)

            # ---- iterations ----
            for it in range(1, ITERS + 1):
                last = it == ITERS
                for img in range(NIMG):
                    zprev = zt[img][(it - 1) % 2]
                    znext = zt[img][it % 2]
                    for g in range(NG):
                        off = CHUNK * g
                        ssl = st[img][:, off:off + CHUNK]
                        ps = psp.tile([128, CHUNK], F32, tag="ps")
                        if not last:
                            conv_group(ps, 2, zrhs(zprev, g), g)
                            nc.vector.scalar_tensor_tensor(
                                out=ssl, in0=ssl, scalar=0.9, in1=ps[:],
                                op0=mybir.AluOpType.mult, op1=mybir.AluOpType.add)
                            nc.vector.tensor_tensor(
                                ssl, ssl, ict[img][:, off:off + CHUNK],
                                mybir.AluOpType.add)
                        else:
                            # fold "+ict" into psum via identity matmuls so the
                            # tail only needs one DVE op per chunk
                            def ic_rhs(b, img=img, off=off):
                                return ict[img][:, off + 512 * b:off + 512 * b + 512]
                            conv_group(ps, 2, zrhs(zprev, g), g, ic_rhs=ic_rhs)
                        if not last:
                            u = scr.tile([128, CHUNK], F32, tag="u")
                            nc.scalar.activation(u[:], ssl, LR,
                                                 bias=b1[:], scale=-1.0, alpha=ALPHA)
                            nc.scalar.activation(
                                znext[:, 1 + 32 * g:33 + 32 * g, 1:65], u[:], LR,
                                bias=b2[:], scale=-1.0, alpha=ALPHA)
                        else:
                            # half-chunk pipeline to shorten the kernel tail;
                            # write z-space result, host subtracts the 1
                            u = scr.tile([128, CHUNK], F32, tag="u")
                            for h in range(2):
                                ho, hsl = 1024 * h, slice(1024 * h, 1024 * h + 1024)
                                nc.vector.scalar_tensor_tensor(
                                    out=st[img][:, off + ho:off + ho + 1024],
                                    in0=st[img][:, off + ho:off + ho + 1024],
                                    scalar=0.9, in1=ps[:, hsl],
                                    op0=mybir.AluOpType.mult,
                                    op1=mybir.AluOpType.add)
                                nc.scalar.activation(
                                    u[:, hsl], st[img][:, off + ho:off + ho + 1024],
                                    LR, bias=b1[:], scale=-1.0, alpha=ALPHA)
                                nc.scalar.activation(u[:, hsl], u[:, hsl], LR,
                                                     bias=b2[:], scale=-1.0,
                                                     alpha=ALPHA)
                                nc.sync.dma_start(
                                    yo_d[:, img, off + ho:off + ho + 1024],
                                    u[:, hsl])

    nc.compile()
    return nc


def pack_inputs(x, rescale_w, rescale_b, A_w, A_b, B_w, B_b, Z, n_cores=N_CORES):
    """Host-side prep: parity-pack x per core, build fp16 lhsT blocks, biases."""
    x = np.asarray(x, dtype=np.float32)

    def blocks(w):  # w [o,c,3,3] f32 -> [128, 12*64] f16 lhsT blocks
        out = np.zeros((128, 12 * 64), dtype=np.float16)
        for di, dy in enumerate((-1, 0, 1)):
            b0 = di * 4 * 64
            c = w[:, :, dy + 1, 1].T.astype(np.float16)  # dx=0
            l = w[:, :, dy + 1, 0].T.astype(np.float16)  # dx=-1
            r = w[:, :, dy + 1, 2].T.astype(np.float16)  # dx=+1
            out[0:64, b0:b0 + 64] = c          # T0 dense: even data, dx=0
            out[64:128, b0:b0 + 64] = r        # T0 dense: odd data, dx=+1
            out[0:64, b0 + 64:b0 + 128] = l    # T1 dense: even data, dx=-1
            out[64:128, b0 + 64:b0 + 128] = c  # T1 dense: odd data, dx=0
            out[64:128, b0 + 128:b0 + 192] = l  # T0 edge: odd data @ j-1, dx=-1
            out[0:64, b0 + 192:b0 + 256] = r    # T1 edge: even data @ j+1, dx=+1
        return out

    ident = np.zeros((128, 2 * 64), dtype=np.float16)
    ident[0:64, 0:64] = np.eye(64, dtype=np.float16)      # T0: psum[m] += rhs[m]
    ident[64:128, 64:128] = np.eye(64, dtype=np.float16)  # T1: psum[64+m] += rhs[64+m]
    wt = np.concatenate([
        blocks(np.asarray(rescale_w, np.float32)),
        blocks(0.1 * np.asarray(B_w, np.float32)),
        blocks(0.1 * np.asarray(A_w, np.float32)),
        ident,
    ], axis=1)

    # vb = 0.1(B_b+Z+A_b) - CA;  CA = per-out-channel sum of fp16 A taps used
    A16 = (0.1 * np.asarray(A_w, np.float32)).astype(np.float16).astype(np.float32)
    CA = A16.sum(axis=(1, 2, 3))
    vb = (0.1 * (np.asarray(B_b) + np.asarray(Z) + np.asarray(A_b)) - CA).astype(np.float32)
    bias = np.zeros((128, 2), dtype=np.float32)
    bias[0:64, 0] = rescale_b
    bias[64:128, 0] = rescale_b
    bias[0:64, 1] = vb
    bias[64:128, 1] = vb

    in_maps = []
    for c in range(n_cores):
        xp = np.zeros((128, NIMG, ROWS, PITCH), dtype=np.float16)
        for i in range(NIMG):
            g = x[c * NIMG + i]  # [64, 128, 128]
            xp[0:64, i, 1:129, 1:65] = g[:, :, 0::2]
            xp[64:128, i, 1:129, 1:65] = g[:, :, 1::2]
        in_maps.append({"xp": xp, "wt": wt, "bias": bias})
    return in_maps


def unpack_outputs(results, n_cores=N_CORES):
    out = np.empty((n_cores * NIMG, 64, 128, 128), dtype=np.float32)
    for c in range(n_cores):
        yo = results[c]["yo"].reshape(128, NIMG, 128, 64)
        for i in range(NIMG):
            # device stores z-space (nonlin + 1); undo the shift here
            out[c * NIMG + i, :, :, 0::2] = yo[0:64, i] - 1.0
            out[c * NIMG + i, :, :, 1::2] = yo[64:128, i] - 1.0
    return out


def kernel(x, rescale_w, rescale_b, A_w, A_b, B_w, B_b, Z, **_):
    global _NC_CACHE
    if _NC_CACHE is None:
        _NC_CACHE = build_nc()
    in_maps = pack_inputs(x, rescale_w, rescale_b, A_w, A_b, B_w, B_b, Z)
    res = run_bass_kernel_spmd(_NC_CACHE, in_maps, list(range(N_CORES)))
    return unpack_outputs(res.results)
